# revision 1
# baseline (speedup 1.0000x reference)
"""GQA attention (dense_transformer) on 8 TRN2 NeuronCores.

Sharding: core c = b*4 + j  (b = batch 0..1, j = tensor-parallel rank 0..3).
Each core computes q-heads 8j..8j+7 (kv heads 2j, 2j+1) for batch b, then an
AllGather of attn^T over the 4 ranks of its batch group, then its 512-column
shard of the output projection.  Host assembles the full output.

Structure: projection t-chunks of 256 (SBUF residency), attention/AllGather/
wo windows of 512 (amortizes per-op overheads).  Causal diagonal tiles are
subranged (only the valid tq range is computed) plus one 128x128 triangle
mask.  All big matmuls run in float32r (full-rate PE, ~1e-4 rel precision);
the AllGather + wo tail runs in bf16.

Self-contained: hardcodes shapes from the problem spec.
"""
import os
import sys

sys.path.insert(0, "/opt/trn_rl_repo")

from contextlib import ExitStack

import numpy as np
import ml_dtypes

import concourse.bass as bass
import concourse.mybir as mybir
import concourse.tile as tile
from concourse import bacc
from concourse.bass_utils import run_bass_kernel_spmd
from concourse.masks import make_identity

HIDDEN = 2048
N_HEADS = 32
N_KV_HEADS = 8
HEAD_DIM = 64
B_FULL, T_FULL = 2, 2048

NCORES = 8
NTP = 4                       # tensor-parallel ranks per batch group
NHL = N_HEADS // NTP          # 8 local q heads
NKVL = N_KV_HEADS // NTP      # 2 local kv heads
QF = NHL * HEAD_DIM           # 512 local q features
KF = NKVL * HEAD_DIM          # 128 local kv features
COLS = HIDDEN // NTP          # 512 output columns per rank
TCP = 256                     # projection t-chunk width
TCA = 512                     # attention window width
P = 128

F32 = mybir.dt.float32
F32R = mybir.dt.float32r
BF16 = mybir.dt.bfloat16

SCALE = 1.0 / np.sqrt(HEAD_DIM)

LAST_EXEC_NS = None
LAST_RESULTS = None


def build_kernel(T=T_FULL, repeat=1, no_ag=False, ag_mode='full8'):
    """One SPMD program; every core runs the same code on its shard."""
    assert T % TCA == 0
    NW = T // TCA             # attention windows
    KH = HIDDEN // P          # 16 k-tiles over hidden
    NTT = T // P              # tk tiles total
    WTK = TCA // P            # tk tiles per window (4)

    nc = bacc.Bacc("TRN2", debug=False)

    xT = nc.dram_tensor("xT", [HIDDEN, T], F32R, kind="ExternalInput")
    wqT = nc.dram_tensor("wqT", [HIDDEN, QF], F32R, kind="ExternalInput")
    wkT = nc.dram_tensor("wkT", [HIDDEN, KF], F32R, kind="ExternalInput")
    wvT = nc.dram_tensor("wvT", [HIDDEN, KF], F32R, kind="ExternalInput")
    woT = nc.dram_tensor("woT", [2 * HIDDEN, COLS], BF16, kind="ExternalInput")
    cosT = nc.dram_tensor("cosT", [P, T], F32R, kind="ExternalInput")
    sinTs = nc.dram_tensor("sinTs", [P, T], F32R, kind="ExternalInput")
    swp = nc.dram_tensor("swp", [P, P], F32R, kind="ExternalInput")
    msk = nc.dram_tensor("msk", [P, P], F32R, kind="ExternalInput")
    out = nc.dram_tensor("out", [COLS, T], F32, kind="ExternalOutput")

    n_gather = NCORES if ag_mode in ('full8', 'single8') else NTP
    if ag_mode == 'single8':
        cc_in = [nc.dram_tensor(f"cc_in{i}", [QF, T], BF16)
                 for i in range(repeat)]
        cc_out = [nc.dram_tensor(f"cc_out{i}", [n_gather * QF, T], BF16)
                  for i in range(repeat)]
    else:
        GSZ = 2 if NW % 2 == 0 else 1      # windows gathered per collective
        NPAIR = NW // GSZ
        cc_in = [nc.dram_tensor(f"cc_in{i}", [QF, GSZ * TCA], BF16)
                 for i in range(NPAIR * repeat)]
        cc_out = [nc.dram_tensor(f"cc_out{i}", [n_gather * QF, GSZ * TCA], BF16)
                  for i in range(NPAIR * repeat)]
    groups = ([[0, 1, 2, 3, 4, 5, 6, 7]] if ag_mode == 'full8'
              else [[0, 1, 2, 3], [4, 5, 6, 7]])

    with tile.TileContext(nc) as tc, ExitStack() as est:
        consts = est.enter_context(tc.tile_pool(name="consts", bufs=1))
        kpool = est.enter_context(tc.tile_pool(name="kpool", bufs=1))
        xcpool = est.enter_context(tc.tile_pool(name="xcpool", bufs=17))
        stream = est.enter_context(tc.tile_pool(name="stream", bufs=3))
        qrpool = est.enter_context(tc.tile_pool(name="qrpool", bufs=6))
        ppool = est.enter_context(tc.tile_pool(name="ppool", bufs=4))
        atpool = est.enter_context(tc.tile_pool(name="atpool", bufs=6))
        agpool = est.enter_context(tc.tile_pool(name="agpool", bufs=32))
        small = est.enter_context(tc.tile_pool(name="small", bufs=2))
        ps_proj = est.enter_context(tc.tile_pool(name="ps_proj", bufs=2, space="PSUM"))
        ps_s = est.enter_context(tc.tile_pool(name="ps_s", bufs=2, space="PSUM"))
        ps_pv = est.enter_context(tc.tile_pool(name="ps_pv", bufs=2, space="PSUM"))
        ps_y = est.enter_context(tc.tile_pool(name="ps_y", bufs=1, space="PSUM"))
        ps_misc = est.enter_context(tc.tile_pool(name="ps_misc", bufs=1, space="PSUM"))

        # ---- constants (DMA order matters for startup: weights first, then
        # rope tables, mask, wo) ----
        swp_sb = consts.tile([P, P], F32R)
        wq_sb = consts.tile([P, KH, QF], F32R)
        wk_sb = consts.tile([P, KH, KF], F32R)
        wv_sb = consts.tile([P, KH, KF], F32R)
        wo_sb = consts.tile([P, 2 * KH, COLS], BF16)
        cos_sb = consts.tile([P, 2, TCA], F32R)
        sin_sb = consts.tile([P, 2, TCA], F32R)
        msk_sb = consts.tile([P, P], F32R)
        id_sb = consts.tile([P, P], F32R)
        id_f32 = consts.tile([P, P], F32)
        ones_sb = consts.tile([1, HEAD_DIM], F32R)
        ones_f32 = consts.tile([P, 1], F32)
        ones_row_f32 = consts.tile([1, HEAD_DIM], F32)

        xv = xT[:, :].rearrange("(t p) n -> p t n", p=P)
        nc.sync.dma_start(out=swp_sb, in_=swp[:, :])
        wqv = wqT[:, :].rearrange("(t p) f -> p t f", p=P)
        wkv = wkT[:, :].rearrange("(t p) f -> p t f", p=P)
        wvv = wvT[:, :].rearrange("(t p) f -> p t f", p=P)
        wov = woT[:, :].rearrange("(t p) f -> p t f", p=P)
        xc0 = []
        for k in range(KH):
            nc.sync.dma_start(out=wq_sb[:, k, :], in_=wqv[:, k, :])
            xt_ = xcpool.tile([P, TCP], F32R, tag="xc")
            nc.sync.dma_start(out=xt_, in_=xv[:, k, 0:TCP])
            xc0.append(xt_)
        for k in range(KH):
            nc.sync.dma_start(out=wk_sb[:, k, :], in_=wkv[:, k, :])
            nc.sync.dma_start(out=wv_sb[:, k, :], in_=wvv[:, k, :])
        sl = slice(0, TCA)
        nc.sync.dma_start(out=cos_sb[:, 0, :], in_=cosT[:, sl])
        nc.sync.dma_start(out=sin_sb[:, 0, :], in_=sinTs[:, sl])
        nc.sync.dma_start(out=msk_sb, in_=msk[:, :])

        make_identity(nc, id_f32)
        nc.vector.tensor_copy(id_sb, id_f32)
        nc.vector.memset(ones_f32, 1.0)
        nc.vector.memset(ones_row_f32, 1.0)
        nc.vector.tensor_copy(ones_sb, ones_row_f32)

        # ---- persistent K / V accumulators ----
        KA = kpool.tile([P, T], F32R, tag="KA")   # [g0; g0] roped K^T
        KB = kpool.tile([P, T], F32R, tag="KB")   # [g1; g1]
        # V natural layout per tk-tile: cols = [V_g0 (64) | 1 | V_g1 (64) | 1]
        vaug = kpool.tile([P, NTT, 2 * HEAD_DIM + 2], F32R, tag="vaug")
        for t in range(NTT):
            nc.vector.tensor_copy(vaug[:, t, HEAD_DIM:HEAD_DIM + 1], ones_f32)
            nc.vector.tensor_copy(vaug[:, t, 2 * HEAD_DIM + 1:2 * HEAD_DIM + 2],
                                  ones_f32)


        def rope(raw_sb, cs, ss, out_ap):
            """out = raw*cos + swap(raw)*sin_signed  (all [P, TCP])."""
            sw_ps = ps_misc.tile([P, TCP], F32, tag="misc")
            nc.tensor.matmul(sw_ps, lhsT=swp_sb, rhs=raw_sb, start=True, stop=True)
            m2 = stream.tile([P, TCP], F32R, tag="tmp")
            nc.vector.tensor_tensor(out=m2, in0=sw_ps, in1=ss, op=mybir.AluOpType.mult)
            nc.vector.tensor_tensor(out=out_ap, in0=raw_sb, in1=cs,
                                    op=mybir.AluOpType.mult)
            nc.vector.tensor_tensor(out=out_ap, in0=out_ap, in1=m2,
                                    op=mybir.AluOpType.add)

        def project(c, qrope, half, xc=None):
            """Projections + rope for t-chunk c; writes qrope[:][half]."""
            csl = slice(c * TCP, (c + 1) * TCP)
            hsl = slice(half * TCP, (half + 1) * TCP)
            slot = (c // 2) % 2
            lsl_c = slice((c % 2) * TCP, (c % 2 + 1) * TCP)
            cs = cos_sb[:, slot, lsl_c]
            ss = sin_sb[:, slot, lsl_c]
            if xc is None:
                xc = []
                for k in range(KH):
                    t_ = xcpool.tile([P, TCP], F32R, tag="xc")
                    nc.sync.dma_start(out=t_, in_=xv[:, k, csl])
                    xc.append(t_)
            # Q: 4 head-pair tiles
            for m in range(4):
                q_ps = ps_proj.tile([P, TCP], F32, tag="proj")
                for k in range(KH):
                    nc.tensor.matmul(q_ps, lhsT=wq_sb[:, k, m * P:(m + 1) * P],
                                     rhs=xc[k], start=(k == 0), stop=(k == KH - 1))
                raw = stream.tile([P, TCP], F32R, tag="raw")
                nc.vector.tensor_copy(raw, q_ps)
                rope(raw, cs, ss, qrope[m][:, hsl])
            # K
            k_ps = ps_proj.tile([P, TCP], F32, tag="proj")
            for k in range(KH):
                nc.tensor.matmul(k_ps, lhsT=wk_sb[:, k, :], rhs=xc[k],
                                 start=(k == 0), stop=(k == KH - 1))
            kraw = stream.tile([P, TCP], F32R, tag="raw")
            nc.vector.tensor_copy(kraw, k_ps)
            krope = stream.tile([P, TCP], F32R, tag="raw")
            rope(kraw, cs, ss, krope)
            nc.vector.tensor_copy(KA[0:64, csl], krope[0:64, :])
            nc.vector.tensor_copy(KA[64:128, csl], krope[0:64, :])
            nc.vector.tensor_copy(KB[0:64, csl], krope[64:128, :])
            nc.vector.tensor_copy(KB[64:128, csl], krope[64:128, :])
            # V (as V^T) then transpose into vaug
            v_ps = ps_proj.tile([P, TCP], F32, tag="proj")
            for k in range(KH):
                nc.tensor.matmul(v_ps, lhsT=wv_sb[:, k, :], rhs=xc[k],
                                 start=(k == 0), stop=(k == KH - 1))
            vt = stream.tile([P, TCP], F32R, tag="raw")
            nc.vector.tensor_copy(vt, v_ps)
            for tt in range(TCP // P):
                tp_ps = ps_misc.tile([P, P], F32R, tag="misc")
                nc.tensor.transpose(tp_ps, vt[:, tt * P:(tt + 1) * P], id_sb)
                tkt = c * (TCP // P) + tt
                nc.vector.tensor_copy(vaug[:, tkt, 0:HEAD_DIM], tp_ps[:, 0:HEAD_DIM])
                nc.vector.tensor_copy(vaug[:, tkt, HEAD_DIM + 1:2 * HEAD_DIM + 1],
                                      tp_ps[:, HEAD_DIM:2 * HEAD_DIM])

        NKW = 2 * KH if ag_mode == 'full8' else KH

        def emit_wo(pi, w0):
            ccv = cc_out[pi][:, :].rearrange("(t p) n -> p t n", p=P)
            for sw in range(GSZ):
                ssl = slice(sw * TCA, (sw + 1) * TCA)
                osl = slice((w0 + sw) * TCA, (w0 + sw + 1) * TCA)
                ag = []
                for k in range(NKW):
                    ag_t = agpool.tile([P, TCA], BF16, tag="ag")
                    nc.sync.dma_start(out=ag_t, in_=ccv[:, k, ssl])
                    ag.append(ag_t)
                for m in range(4):
                    y_ps = ps_y.tile([P, TCA], F32, tag="y")
                    for k in range(NKW):
                        nc.tensor.matmul(y_ps, lhsT=wo_sb[:, k, m * P:(m + 1) * P],
                                         rhs=ag[k], start=(k == 0),
                                         stop=(k == NKW - 1))
                    y_sb = small.tile([P, TCA], F32, tag="ysb")
                    nc.vector.tensor_copy(y_sb, y_ps)
                    nc.sync.dma_start(out=out[m * P:(m + 1) * P, osl], in_=y_sb)

        pending_wo = None
        for rep in range(repeat):
            for w in range(NW):
                wsl = slice(w * TCA, (w + 1) * TCA)
                qrope = []
                for _ in range(4):
                    qr_t = qrpool.tile([P, TCA], F32R, tag="qrope")
                    qrope.append(qr_t)
                first = (rep == 0 and w == 0)
                project(2 * w, qrope, 0, xc=xc0 if first else None)
                project(2 * w + 1, qrope, 1)

                # ---- attention window ----
                n_tk = (w + 1) * WTK
                at_tiles = []
                for _ in range(4):
                    at_t = atpool.tile([P, TCA], BF16, tag="attnT")
                    at_tiles.append(at_t)
                for h in range(NHL):
                    g = h // (NHL // NKVL)
                    par = h % 2
                    base = par * HEAD_DIM
                    ksrc = KA if g == 0 else KB
                    qt = qrope[h // 2]
                    lsl = slice(base, base + HEAD_DIM)

                    pv_ps = ps_pv.tile([HEAD_DIM + 1, TCA], F32, tag="pv")
                    for i in range(n_tk):
                        o = i - w * WTK
                        lo = max(o, 0) * P
                        s_ps = ps_s.tile([P, TCA], F32, tag="s")
                        nc.tensor.matmul(
                            s_ps[:, lo:],
                            lhsT=ksrc[lsl, i * P:(i + 1) * P],
                            rhs=qt[lsl, lo:],
                            start=True, stop=True)
                        p_sb = ppool.tile([P, TCA], F32R, tag="p")
                        nc.scalar.activation(out=p_sb[:, lo:], in_=s_ps[:, lo:],
                                             func=mybir.ActivationFunctionType.Exp,
                                             scale=float(SCALE))
                        if o >= 0:
                            nc.vector.tensor_tensor(out=p_sb[:, lo:lo + P],
                                                    in0=p_sb[:, lo:lo + P],
                                                    in1=msk_sb,
                                                    op=mybir.AluOpType.mult)
                        vsl = slice(g * (HEAD_DIM + 1), (g + 1) * (HEAD_DIM + 1))
                        nc.tensor.matmul(pv_ps[:, lo:], lhsT=vaug[:, i, vsl],
                                         rhs=p_sb[:, lo:],
                                         start=(i == 0), stop=(i == n_tk - 1))

                    rec = small.tile([1, TCA], F32R, tag="recip")
                    with nc.allow_low_precision(reason="f32r softmax denom"):
                        nc.vector.reciprocal(rec, pv_ps[HEAD_DIM:HEAD_DIM + 1, :])
                    rep_ps = ps_misc.tile([HEAD_DIM, TCA], F32, tag="misc")
                    nc.tensor.matmul(rep_ps, lhsT=ones_sb, rhs=rec,
                                     start=True, stop=True)
                    rep_sb = small.tile([HEAD_DIM, TCA], F32, tag="rep")
                    nc.scalar.activation(out=rep_sb, in_=rep_ps,
                                         func=mybir.ActivationFunctionType.Copy)
                    nc.vector.tensor_tensor(
                        out=at_tiles[h // 2][base:base + HEAD_DIM, :],
                        in0=pv_ps[0:HEAD_DIM, :], in1=rep_sb,
                        op=mybir.AluOpType.mult)

                    if h == 3 and pending_wo is not None:
                        emit_wo(*pending_wo)
                        pending_wo = None

                # ---- AllGather attn^T window across the 4 TP ranks ----
                pi = rep * (NW // GSZ) + w // GSZ
                psl = slice((w % GSZ) * TCA, (w % GSZ + 1) * TCA)
                for m in range(4):
                    nc.sync.dma_start(out=cc_in[pi][m * P:(m + 1) * P, psl],
                                      in_=at_tiles[m])
                if w % GSZ == GSZ - 1:
                    nc.gpsimd.collective_compute(
                        "AllGather", mybir.AluOpType.bypass,
                        replica_groups=groups,
                        ins=[cc_in[pi][:, :]],
                        outs=[cc_out[pi][:, :]],
                    )
                    pending_wo = (pi, w - GSZ + 1)

                if rep == 0 and w == 0:
                    for k in range(2 * KH):
                        nc.sync.dma_start(out=wo_sb[:, k, :], in_=wov[:, k, :])
                if not (w + 1 == NW and rep + 1 == repeat):
                    nw_ = (w + 1) % NW
                    nsl = slice(nw_ * TCA, (nw_ + 1) * TCA)
                    nslot = (w + 1) % 2
                    nc.sync.dma_start(out=cos_sb[:, nslot, :], in_=cosT[:, nsl])
                    nc.sync.dma_start(out=sin_sb[:, nslot, :], in_=sinTs[:, nsl])

        if pending_wo is not None:
            emit_wo(*pending_wo)

    nc.compile()
    return nc


_NC_CACHE = {}


def _get_nc(T):
    if T not in _NC_CACHE:
        _NC_CACHE[T] = build_kernel(T)
    return _NC_CACHE[T]


def _perm64():
    """Per-head permutation: interleaved (even,odd) -> [r(32) | i(32)]."""
    p = np.empty(HEAD_DIM, dtype=np.int64)
    p[:32] = np.arange(0, HEAD_DIM, 2)
    p[32:] = np.arange(1, HEAD_DIM, 2)
    return p


def make_inputs(x, freqs_cis, wq, wk, wv, wo, T):
    """Build the 8 per-core input maps (host-side sharding + layout prep)."""
    perm = _perm64()
    f32 = np.float32

    cos = np.asarray(freqs_cis[:T, :, 0], dtype=f32)   # [T, 32]
    sin = np.asarray(freqs_cis[:T, :, 1], dtype=f32)
    cosT = np.tile(cos.T, (4, 1)).astype(f32)                        # [128, T]
    sinTs = np.tile(np.vstack([-sin.T, sin.T]), (2, 1)).astype(f32)  # [128, T]

    J = np.zeros((HEAD_DIM, HEAD_DIM), dtype=f32)
    J[np.arange(32), np.arange(32) + 32] = 1.0
    J[np.arange(32) + 32, np.arange(32)] = 1.0
    swp = np.zeros((P, P), dtype=f32)
    swp[:HEAD_DIM, :HEAD_DIM] = J
    swp[HEAD_DIM:, HEAD_DIM:] = J

    # single causal triangle mask [128, 128]: msk[p, q] = (q >= p)
    q_idx = np.arange(P)
    p_idx = np.arange(P)[:, None]
    msk = (q_idx[None, :] >= p_idx).astype(f32)

    def permute_heads(w, n_heads):
        wh = np.asarray(w, f32).reshape(n_heads, HEAD_DIM, HIDDEN)
        return wh[:, perm, :].reshape(n_heads * HEAD_DIM, HIDDEN)

    wq_p = permute_heads(wq, N_HEADS)
    wk_p = permute_heads(wk, N_KV_HEADS)
    wv_n = np.asarray(wv, f32)
    wo_n = np.asarray(wo, f32)

    in_maps = []
    for core in range(NCORES):
        b, j = divmod(core, NTP)
        xTc = np.ascontiguousarray(np.asarray(x[b, :T], f32).T)     # [H, T]
        wqTc = np.ascontiguousarray(wq_p[j * QF:(j + 1) * QF].T)    # [H, QF]
        wkTc = np.ascontiguousarray(wk_p[j * KF:(j + 1) * KF].T)
        wvTc = np.ascontiguousarray(wv_n[j * KF:(j + 1) * KF].T)
        woTc = np.zeros((2 * HIDDEN, COLS), dtype=ml_dtypes.bfloat16)
        woTc[b * HIDDEN:(b + 1) * HIDDEN] = wo_n[j * COLS:(j + 1) * COLS].T.astype(
            ml_dtypes.bfloat16)                  # own-batch rows only
        in_maps.append({
            "xT": xTc, "wqT": wqTc, "wkT": wkTc, "wvT": wvTc, "woT": woTc,
            "cosT": cosT, "sinTs": sinTs, "swp": swp, "msk": msk,
        })
    return in_maps


def kernel(x, freqs_cis, wq, wk, wv, wo):
    global LAST_EXEC_NS, LAST_RESULTS
    T = x.shape[1]
    nc = _get_nc(T)
    in_maps = make_inputs(x, freqs_cis, wq, wk, wv, wo, T)
    trace = bool(int(os.environ.get("KERNEL_TRACE", "0")))
    res = run_bass_kernel_spmd(nc, in_maps, core_ids=list(range(NCORES)),
                               trace=trace)
    LAST_EXEC_NS = res.exec_time_ns
    LAST_RESULTS = res
    out = np.empty((B_FULL, T, HIDDEN), dtype=np.float32)
    for core in range(NCORES):
        b, j = divmod(core, NTP)
        out[b, :, j * COLS:(j + 1) * COLS] = res.results[core]["out"].T
    return out



# revision 11
# speedup vs baseline: 1.2687x; 1.2687x over previous
"""GQA attention (dense_transformer) on 8 TRN2 NeuronCores.

Sharding: core c = b*4 + j  (b = batch 0..1, j = tensor-parallel rank 0..3).
Each core computes q-heads 8j..8j+7 (kv heads 2j, 2j+1) for batch b, then a
collective of attn^T, then its shard of the output projection.

v2 structure vs v1:
  - attention runs per head-PAIR: scores for both heads land in one
    [128, 2*TCA] PSUM tile -> ONE exp activation per k-tile (halves the
    fixed per-instruction ACT overhead that made the attention chain
    ACT-latency-bound).
  - projections of window w+1 and the wo matmuls of the gathered window
    pair are emitted as "filler" pieces interleaved into the attention
    loop, so PE (in-order) has work while exp stalls the score->PV chain.
  - collective mode selectable: full8 AllGather (Shared out) or tp4
    group AllGather (half wire + half wo work).
"""
import os
import sys

sys.path.insert(0, "/opt/trn_rl_repo")

from contextlib import ExitStack

import numpy as np
import ml_dtypes

import concourse.bass as bass
import concourse.mybir as mybir
import concourse.tile as tile
from concourse import bacc
from concourse.bass_utils import run_bass_kernel_spmd
from concourse.masks import make_identity

HIDDEN = 2048
N_HEADS = 32
N_KV_HEADS = 8
HEAD_DIM = 64
B_FULL, T_FULL = 2, 2048

NCORES = 8
NTP = 4                       # tensor-parallel ranks per batch group
NHL = N_HEADS // NTP          # 8 local q heads
NKVL = N_KV_HEADS // NTP      # 2 local kv heads
QF = NHL * HEAD_DIM           # 512 local q features
KF = NKVL * HEAD_DIM          # 128 local kv features
COLS = HIDDEN // NTP          # 512 output columns per rank
TCP = 256                     # projection t-chunk width
TCA = 512                     # attention window width
P = 128

F32 = mybir.dt.float32
F32R = mybir.dt.float32r
BF16 = mybir.dt.bfloat16

SCALE = 1.0 / np.sqrt(HEAD_DIM)

LAST_EXEC_NS = None
LAST_RESULTS = None

AG_MODE = 'tp4'


def build_kernel(T=T_FULL, repeat=1, no_ag=False, ag_mode=AG_MODE):
    """One SPMD program; every core runs the same code on its shard."""
    assert T % TCA == 0
    NW = T // TCA             # attention windows
    KH = HIDDEN // P          # 16 k-tiles over hidden
    NTT = T // P              # tk tiles total
    WTK = TCA // P            # tk tiles per window (4)

    nc = bacc.Bacc("TRN2", debug=False)

    NKW = 2 * KH if ag_mode in ('full8', 'full8s', 'single8') else KH

    xT = nc.dram_tensor("xT", [HIDDEN, T], BF16, kind="ExternalInput")
    wqT = nc.dram_tensor("wqT", [HIDDEN, QF], BF16, kind="ExternalInput")
    wkT = nc.dram_tensor("wkT", [HIDDEN, KF], BF16, kind="ExternalInput")
    wvT = nc.dram_tensor("wvT", [HIDDEN, KF], BF16, kind="ExternalInput")
    woT = nc.dram_tensor("woT", [NKW * P, COLS], BF16, kind="ExternalInput")
    cosT = nc.dram_tensor("cosT", [P, T], BF16, kind="ExternalInput")
    sinTs = nc.dram_tensor("sinTs", [P, T], BF16, kind="ExternalInput")
    swp = nc.dram_tensor("swp", [P, P], BF16, kind="ExternalInput")
    msk = nc.dram_tensor("msk", [P, P], BF16, kind="ExternalInput")
    out = nc.dram_tensor("out", [COLS, T], F32, kind="ExternalOutput")

    n_gather = NCORES if ag_mode in ('full8', 'full8s', 'single8') else NTP
    shared_kw = {"addr_space": "Shared"} if ag_mode == "full8s" else {}
    GSZ = 2 if NW % 2 == 0 else 1      # windows gathered per collective
    NPAIR = NW // GSZ
    cc_in = [nc.dram_tensor(f"cc_in{i}", [QF, GSZ * TCA], BF16)
             for i in range(NPAIR * repeat)]
    cc_out = [nc.dram_tensor(f"cc_out{i}", [n_gather * QF, GSZ * TCA], BF16,
                             **shared_kw)
              for i in range(NPAIR * repeat)]
    groups = ([[0, 1, 2, 3, 4, 5, 6, 7]] if n_gather == 8
              else [[0, 1, 2, 3], [4, 5, 6, 7]])

    with tile.TileContext(nc) as tc, ExitStack() as est:
        consts = est.enter_context(tc.tile_pool(name="consts", bufs=1))
        kpool = est.enter_context(tc.tile_pool(name="kpool", bufs=1))
        xcpool = est.enter_context(tc.tile_pool(name="xcpool", bufs=4))
        stream = est.enter_context(tc.tile_pool(name="stream", bufs=3))
        qrpool = est.enter_context(tc.tile_pool(name="qrpool", bufs=9))
        ppool = est.enter_context(tc.tile_pool(name="ppool", bufs=2))
        atpool = est.enter_context(tc.tile_pool(name="atpool", bufs=2))
        agpool = est.enter_context(tc.tile_pool(name="agpool", bufs=1 if NKW == 32 else 2))
        small = est.enter_context(tc.tile_pool(name="small", bufs=2))
        # PSUM budget: 8 banks of [128 x 2KB].
        #   ps_s:  2 x [128,1024]f32 = 4 banks (pair scores, double-buffered)
        #   ps_pv: 2 x [65,512]f32   = 2 banks (even+odd head accumulators)
        #   ps_ab: 2 x [128,<=512]   = 2 banks (proj / wo / swap / rec shared)
        ps_s = est.enter_context(tc.tile_pool(name="ps_s", bufs=2, space="PSUM"))
        ps_pv = est.enter_context(tc.tile_pool(name="ps_pv", bufs=2, space="PSUM"))
        ps_ab = est.enter_context(tc.tile_pool(name="ps_ab", bufs=2, space="PSUM"))

        # ---- constants (DMA order matters for startup: weights first, then
        # rope tables, mask, wo) ----
        swp_sb = consts.tile([P, P], BF16)
        wq_sb = consts.tile([P, KH, QF], BF16)
        wk_sb = consts.tile([P, KH, KF], BF16)
        wv_sb = consts.tile([P, KH, KF], BF16)
        wo_sb = consts.tile([P, NKW, COLS], BF16)
        cos_sb = consts.tile([P, 2, TCA], BF16)
        sin_sb = consts.tile([P, 2, TCA], BF16)
        msk_sb = consts.tile([P, P], BF16)
        id_sb = consts.tile([P, P], BF16)
        id_f32 = consts.tile([P, P], F32)
        ones_sb = consts.tile([1, HEAD_DIM], F32R)
        ones_f32 = consts.tile([P, 1], F32)
        ones_row_f32 = consts.tile([1, HEAD_DIM], F32)

        xv = xT[:, :].rearrange("(t p) n -> p t n", p=P)
        nc.sync.dma_start(out=swp_sb, in_=swp[:, :])
        wqv = wqT[:, :].rearrange("(t p) f -> p t f", p=P)
        wkv = wkT[:, :].rearrange("(t p) f -> p t f", p=P)
        wvv = wvT[:, :].rearrange("(t p) f -> p t f", p=P)
        wov = woT[:, :].rearrange("(t p) f -> p t f", p=P)
        xc0 = xcpool.tile([P, KH, TCP], BF16, tag="xc")
        nc.sync.dma_start(out=xc0, in_=xv[:, :, 0:TCP])
        nc.sync.dma_start(out=wq_sb, in_=wqv[:, :, :])
        nc.sync.dma_start(out=wk_sb, in_=wkv[:, :, :])
        nc.sync.dma_start(out=wv_sb, in_=wvv[:, :, :])
        sl = slice(0, TCA)
        nc.sync.dma_start(out=cos_sb[:, 0, :], in_=cosT[:, sl])
        nc.sync.dma_start(out=sin_sb[:, 0, :], in_=sinTs[:, sl])
        nc.sync.dma_start(out=msk_sb, in_=msk[:, :])

        make_identity(nc, id_f32)
        nc.vector.tensor_copy(id_sb, id_f32)
        nc.vector.memset(ones_f32, 1.0)
        nc.vector.memset(ones_row_f32, 1.0)
        nc.vector.tensor_copy(ones_sb, ones_row_f32)

        # ---- persistent K / V accumulators ----
        KA = kpool.tile([P, T], BF16, tag="KA")   # [g0; g0] roped K^T
        KB = kpool.tile([P, T], BF16, tag="KB")   # [g1; g1]
        # V natural layout per tk-tile: cols = [V_g0 (64) | 1 | V_g1 (64) | 1]
        vaug = kpool.tile([P, NTT, 2 * HEAD_DIM + 2], BF16, tag="vaug")
        for t in range(NTT):
            nc.vector.tensor_copy(vaug[:, t, HEAD_DIM:HEAD_DIM + 1], ones_f32)
            nc.vector.tensor_copy(vaug[:, t, 2 * HEAD_DIM + 1:2 * HEAD_DIM + 2],
                                  ones_f32)

        def rope(raw_sb, cs, ss, out_ap):
            """out = raw*cos + swap(raw)*sin_signed  (all [P, TCP])."""
            sw_ps = ps_ab.tile([P, TCP], F32, tag="ab")
            nc.tensor.matmul(sw_ps, lhsT=swp_sb, rhs=raw_sb, start=True, stop=True)
            m2 = stream.tile([P, TCP], BF16, tag="tmp")
            nc.vector.tensor_tensor(out=m2, in0=sw_ps, in1=ss, op=mybir.AluOpType.mult)
            nc.vector.tensor_tensor(out=out_ap, in0=raw_sb, in1=cs,
                                    op=mybir.AluOpType.mult)
            nc.vector.tensor_tensor(out=out_ap, in0=out_ap, in1=m2,
                                    op=mybir.AluOpType.add)

        def proj_pieces(c, qrope, half, xc=None):
            """Return filler pieces (closures) for projections of t-chunk c."""
            csl = slice(c * TCP, (c + 1) * TCP)
            hsl = slice(half * TCP, (half + 1) * TCP)
            slot = (c // 2) % 2
            lsl_c = slice((c % 2) * TCP, (c % 2 + 1) * TCP)
            cs = cos_sb[:, slot, lsl_c]
            ss = sin_sb[:, slot, lsl_c]
            if xc is None:
                xc = xcpool.tile([P, KH, TCP], BF16, tag="xc")
                nc.sync.dma_start(out=xc, in_=xv[:, :, csl])

            def q_piece(m):
                def go():
                    q_ps = ps_ab.tile([P, TCP], F32, tag="ab")
                    for k in range(KH):
                        nc.tensor.matmul(q_ps, lhsT=wq_sb[:, k, m * P:(m + 1) * P],
                                         rhs=xc[:, k, :], start=(k == 0),
                                         stop=(k == KH - 1))
                    raw = stream.tile([P, TCP], BF16, tag="raw")
                    nc.vector.tensor_copy(raw, q_ps)
                    rope(raw, cs, ss, qrope[m][:, hsl])
                return go

            def k_piece():
                k_ps = ps_ab.tile([P, TCP], F32, tag="ab")
                for k in range(KH):
                    nc.tensor.matmul(k_ps, lhsT=wk_sb[:, k, :], rhs=xc[:, k, :],
                                     start=(k == 0), stop=(k == KH - 1))
                kraw = stream.tile([P, TCP], BF16, tag="raw")
                nc.vector.tensor_copy(kraw, k_ps)
                krope = stream.tile([P, TCP], BF16, tag="raw")
                rope(kraw, cs, ss, krope)
                nc.vector.tensor_copy(KA[0:64, csl], krope[0:64, :])
                nc.vector.tensor_copy(KA[64:128, csl], krope[0:64, :])
                nc.vector.tensor_copy(KB[0:64, csl], krope[64:128, :])
                nc.vector.tensor_copy(KB[64:128, csl], krope[64:128, :])

            def v_piece():
                v_ps = ps_ab.tile([P, TCP], F32, tag="ab")
                for k in range(KH):
                    nc.tensor.matmul(v_ps, lhsT=wv_sb[:, k, :], rhs=xc[:, k, :],
                                     start=(k == 0), stop=(k == KH - 1))
                vt = stream.tile([P, TCP], BF16, tag="raw")
                nc.vector.tensor_copy(vt, v_ps)
                for tt in range(TCP // P):
                    tp_ps = ps_ab.tile([P, P], BF16, tag="ab")
                    nc.tensor.transpose(tp_ps, vt[:, tt * P:(tt + 1) * P], id_sb)
                    tkt = c * (TCP // P) + tt
                    nc.vector.tensor_copy(vaug[:, tkt, 0:HEAD_DIM],
                                          tp_ps[:, 0:HEAD_DIM])
                    nc.vector.tensor_copy(vaug[:, tkt, HEAD_DIM + 1:2 * HEAD_DIM + 1],
                                          tp_ps[:, HEAD_DIM:2 * HEAD_DIM])

            return ([q_piece(m) for m in range(4)] + [k_piece, v_piece])

        def wo_pieces(pi, w0):
            """Filler pieces for the output projection of gathered pair pi."""
            ccv = cc_out[pi][:, :].rearrange("(t p) n -> p t n", p=P)
            pieces = []
            for sw in range(GSZ):
                ssl = slice(sw * TCA, (sw + 1) * TCA)
                osl = slice((w0 + sw) * TCA, (w0 + sw + 1) * TCA)
                ag = [None]

                def load_ag(ssl=ssl, ag=ag):
                    ag_t = agpool.tile([P, NKW, TCA], BF16, tag="ag")
                    nc.sync.dma_start(out=ag_t, in_=ccv[:, :, ssl])
                    ag[0] = ag_t

                pieces.append(load_ag)
                y4 = small.tile([P, 4, TCA], F32, tag="y4")
                for m in range(4):
                    def mm(m=m, osl=osl, ag=ag, y4=y4):
                        y_ps = ps_ab.tile([P, TCA], F32, tag="ab")
                        for k in range(NKW):
                            nc.tensor.matmul(y_ps,
                                             lhsT=wo_sb[:, k, m * P:(m + 1) * P],
                                             rhs=ag[0][:, k, :], start=(k == 0),
                                             stop=(k == NKW - 1))
                        nc.vector.tensor_copy(y4[:, m, :], y_ps)
                        if m == 3:
                            ov = out[:, :].rearrange("(m p) t -> p m t", p=P)
                            nc.sync.dma_start(out=ov[:, :, osl], in_=y4)
                    pieces.append(mm)
            return pieces

        fillers = []              # queue of (tag, fn) pending filler pieces

        def emit_filler(n=1):
            for _ in range(n):
                if fillers:
                    fillers.pop(0)[1]()

        def flush_tag(tag):
            while any(t == tag for t, _ in fillers):
                fillers.pop(0)[1]()

        for rep in range(repeat):
            for w in range(NW):
                wsl = slice(w * TCA, (w + 1) * TCA)
                first = (rep == 0 and w == 0)
                if first:
                    qrope = []
                    for _ in range(4):
                        qr_t = qrpool.tile([P, TCA], BF16, tag="qrope")
                        qrope.append(qr_t)
                    for p_ in proj_pieces(2 * w, qrope, 0, xc=xc0):
                        p_()
                    for p_ in proj_pieces(2 * w + 1, qrope, 1):
                        p_()
                    qrope_next = None
                else:
                    qrope = qrope_next

                # queue next window's projections as fillers
                if not (w + 1 == NW and rep + 1 == repeat):
                    nw_ = (w + 1) % NW
                    qrope_next = []
                    for _ in range(4):
                        qr_t = qrpool.tile([P, TCA], BF16, tag="qrope")
                        qrope_next.append(qr_t)
                    nsl = slice(nw_ * TCA, (nw_ + 1) * TCA)
                    nslot = (w + 1) % 2
                    nc.sync.dma_start(out=cos_sb[:, nslot, :], in_=cosT[:, nsl])
                    nc.sync.dma_start(out=sin_sb[:, nslot, :], in_=sinTs[:, nsl])
                    gw1 = rep * NW + w + 1
                    fillers.extend((("proj", gw1), p_)
                                   for p_ in proj_pieces(2 * nw_, qrope_next, 0))
                    fillers.extend((("proj", gw1), p_)
                                   for p_ in proj_pieces(2 * nw_ + 1, qrope_next, 1))

                # all proj pieces for THIS window must be emitted before its
                # attention reads qrope/KA/KB/vaug (they may still be queued)
                flush_tag(("proj", rep * NW + w))

                # ---- attention window: 4 head-pairs ----
                n_tk = (w + 1) * WTK
                at4 = atpool.tile([P, 4, TCA], BF16, tag="attnT")
                at_tiles = [at4[:, m_, :] for m_ in range(4)]
                for m in range(4):
                    g = m // 2
                    ksrc = KA if g == 0 else KB
                    qt = qrope[m]
                    vsl = slice(g * (HEAD_DIM + 1), (g + 1) * (HEAD_DIM + 1))

                    pv_e = ps_pv.tile([HEAD_DIM + 1, TCA], F32, tag="pv")
                    pv_o = ps_pv.tile([HEAD_DIM + 1, TCA], F32, tag="pv")
                    for i in range(n_tk):
                        o = i - w * WTK
                        lo = max(o, 0) * P
                        s_ps = ps_s.tile([P, 2 * TCA], F32, tag="s")
                        nc.tensor.matmul(
                            s_ps[:, lo:TCA],
                            lhsT=ksrc[0:HEAD_DIM, i * P:(i + 1) * P],
                            rhs=qt[0:HEAD_DIM, lo:],
                            start=True, stop=True)
                        nc.tensor.matmul(
                            s_ps[:, TCA + lo:],
                            lhsT=ksrc[HEAD_DIM:P, i * P:(i + 1) * P],
                            rhs=qt[HEAD_DIM:P, lo:],
                            start=True, stop=True)
                        p_sb = ppool.tile([P, 2 * TCA], BF16, tag="p")
                        sv = s_ps[:, :].rearrange("p (b c) -> p b c", b=2)
                        pvw = p_sb[:, :].rearrange("p (b c) -> p b c", b=2)
                        nc.scalar.activation(out=pvw[:, :, lo:], in_=sv[:, :, lo:],
                                             func=mybir.ActivationFunctionType.Exp,
                                             scale=float(SCALE))
                        if o >= 0:
                            for b_ in range(2):
                                nc.vector.tensor_tensor(
                                    out=p_sb[:, b_ * TCA + lo:b_ * TCA + lo + P],
                                    in0=p_sb[:, b_ * TCA + lo:b_ * TCA + lo + P],
                                    in1=msk_sb,
                                    op=mybir.AluOpType.mult)
                        nc.tensor.matmul(pv_e[:, lo:], lhsT=vaug[:, i, vsl],
                                         rhs=p_sb[:, lo:TCA],
                                         start=(i == 0), stop=(i == n_tk - 1))
                        nc.tensor.matmul(pv_o[:, lo:], lhsT=vaug[:, i, vsl],
                                         rhs=p_sb[:, TCA + lo:],
                                         start=(i == 0), stop=(i == n_tk - 1))
                        if i % 4 == 3 and i + 1 < n_tk:
                            emit_filler(1)

                    for par, pv_ps in ((0, pv_e), (1, pv_o)):
                        base = par * HEAD_DIM
                        rec = small.tile([1, TCA], F32R, tag="recip")
                        with nc.allow_low_precision(reason="f32r softmax denom"):
                            nc.vector.reciprocal(rec,
                                                 pv_ps[HEAD_DIM:HEAD_DIM + 1, :])
                        rep_ps = ps_ab.tile([HEAD_DIM, TCA], F32, tag="ab")
                        nc.tensor.matmul(rep_ps, lhsT=ones_sb, rhs=rec,
                                         start=True, stop=True)
                        rep_sb = small.tile([HEAD_DIM, TCA], F32, tag="rep")
                        nc.scalar.activation(out=rep_sb, in_=rep_ps,
                                             func=mybir.ActivationFunctionType.Copy)
                        nc.vector.tensor_tensor(
                            out=at_tiles[m][base:base + HEAD_DIM, :],
                            in0=pv_ps[0:HEAD_DIM, :], in1=rep_sb,
                            op=mybir.AluOpType.mult)
                    emit_filler(2 if w >= 2 else 1)

                # ---- collective of attn^T window across ranks ----
                pi = rep * NPAIR + w // GSZ
                psl = slice((w % GSZ) * TCA, (w % GSZ + 1) * TCA)
                civ = cc_in[pi][:, :].rearrange("(m p) t -> p m t", p=P)
                nc.sync.dma_start(out=civ[:, :, psl], in_=at4)
                if w % GSZ == GSZ - 1:
                    if not no_ag:
                        nc.gpsimd.collective_compute(
                            "AllGather", mybir.AluOpType.bypass,
                            replica_groups=groups,
                            ins=[cc_in[pi][:, :]],
                            outs=[cc_out[pi][:, :]],
                        )
                    fillers.extend((("wo", pi), p_)
                                   for p_ in wo_pieces(pi, w - GSZ + 1))

                if rep == 0 and w == 0:
                    for k in range(NKW):
                        nc.sync.dma_start(out=wo_sb[:, k, :], in_=wov[:, k, :])

        while fillers:
            emit_filler(1)

    nc.compile()
    return nc


_NC_CACHE = {}


def _get_nc(T):
    if T not in _NC_CACHE:
        _NC_CACHE[T] = build_kernel(T, ag_mode=AG_MODE)
    return _NC_CACHE[T]


def _perm64():
    """Per-head permutation: interleaved (even,odd) -> [r(32) | i(32)]."""
    p = np.empty(HEAD_DIM, dtype=np.int64)
    p[:32] = np.arange(0, HEAD_DIM, 2)
    p[32:] = np.arange(1, HEAD_DIM, 2)
    return p


def make_inputs(x, freqs_cis, wq, wk, wv, wo, T, ag_mode=None):
    """Build the 8 per-core input maps (host-side sharding + layout prep)."""
    ag_mode = ag_mode or AG_MODE
    perm = _perm64()
    f32 = np.float32

    cos = np.asarray(freqs_cis[:T, :, 0], dtype=f32)   # [T, 32]
    sin = np.asarray(freqs_cis[:T, :, 1], dtype=f32)
    cosT = np.tile(cos.T, (4, 1)).astype(f32)                        # [128, T]
    sinTs = np.tile(np.vstack([-sin.T, sin.T]), (2, 1)).astype(f32)  # [128, T]

    J = np.zeros((HEAD_DIM, HEAD_DIM), dtype=f32)
    J[np.arange(32), np.arange(32) + 32] = 1.0
    J[np.arange(32) + 32, np.arange(32)] = 1.0
    swp = np.zeros((P, P), dtype=f32)
    swp[:HEAD_DIM, :HEAD_DIM] = J
    swp[HEAD_DIM:, HEAD_DIM:] = J

    # single causal triangle mask [128, 128]: msk[p, q] = (q >= p)
    q_idx = np.arange(P)
    p_idx = np.arange(P)[:, None]
    msk = (q_idx[None, :] >= p_idx).astype(f32)

    def permute_heads(w, n_heads):
        wh = np.asarray(w, f32).reshape(n_heads, HEAD_DIM, HIDDEN)
        return wh[:, perm, :].reshape(n_heads * HEAD_DIM, HIDDEN)

    wq_p = permute_heads(wq, N_HEADS)
    wk_p = permute_heads(wk, N_KV_HEADS)
    wv_n = np.asarray(wv, f32)
    wo_n = np.asarray(wo, f32)

    bf16 = ml_dtypes.bfloat16
    in_maps = []
    for core in range(NCORES):
        b, j = divmod(core, NTP)
        xTc = np.ascontiguousarray(np.asarray(x[b, :T], f32).T.astype(bf16))
        wqTc = np.ascontiguousarray(wq_p[j * QF:(j + 1) * QF].T.astype(bf16))
        wkTc = np.ascontiguousarray(wk_p[j * KF:(j + 1) * KF].T.astype(bf16))
        wvTc = np.ascontiguousarray(wv_n[j * KF:(j + 1) * KF].T.astype(bf16))
        wo_own = wo_n[j * COLS:(j + 1) * COLS].T.astype(bf16)
        if ag_mode in ('full8', 'full8s', 'single8'):
            woTc = np.zeros((2 * HIDDEN, COLS), dtype=bf16)
            woTc[b * HIDDEN:(b + 1) * HIDDEN] = wo_own  # own-batch rows only
        else:
            woTc = np.ascontiguousarray(wo_own)
        in_maps.append({
            "xT": xTc, "wqT": wqTc, "wkT": wkTc, "wvT": wvTc, "woT": woTc,
            "cosT": cosT.astype(bf16), "sinTs": sinTs.astype(bf16),
            "swp": swp.astype(bf16), "msk": msk.astype(bf16),
        })
    return in_maps


def kernel(x, freqs_cis, wq, wk, wv, wo):
    global LAST_EXEC_NS, LAST_RESULTS
    T = x.shape[1]
    nc = _get_nc(T)
    in_maps = make_inputs(x, freqs_cis, wq, wk, wv, wo, T)
    trace = bool(int(os.environ.get("KERNEL_TRACE", "0")))
    res = run_bass_kernel_spmd(nc, in_maps, core_ids=list(range(NCORES)),
                               trace=trace)
    LAST_EXEC_NS = res.exec_time_ns
    LAST_RESULTS = res
    out = np.empty((B_FULL, T, HIDDEN), dtype=np.float32)
    for core in range(NCORES):
        b, j = divmod(core, NTP)
        out[b, :, j * COLS:(j + 1) * COLS] = res.results[core]["out"].T
    return out


# revision 12
# speedup vs baseline: 1.4385x; 1.1338x over previous
"""GQA attention (dense_transformer) on 8 TRN2 NeuronCores.

Sharding: core c = b*4 + j  (b = batch 0..1, j = tensor-parallel rank 0..3).
Each core computes q-heads 8j..8j+7 (kv heads 2j, 2j+1) for batch b, then a
collective of attn^T, then its shard of the output projection.

v2 structure vs v1:
  - attention runs per head-PAIR: scores for both heads land in one
    [128, 2*TCA] PSUM tile -> ONE exp activation per k-tile (halves the
    fixed per-instruction ACT overhead that made the attention chain
    ACT-latency-bound).
  - projections of window w+1 and the wo matmuls of the gathered window
    pair are emitted as "filler" pieces interleaved into the attention
    loop, so PE (in-order) has work while exp stalls the score->PV chain.
  - collective mode selectable; default tp4one: a SINGLE 4-rank
    AllGather per rep (collectives here cost ~200us fixed each and do
    not pipeline, so one big gather beats two half-size ones).
"""
import os
import sys

sys.path.insert(0, "/opt/trn_rl_repo")

from contextlib import ExitStack

import numpy as np
import ml_dtypes

import concourse.bass as bass
import concourse.mybir as mybir
import concourse.tile as tile
from concourse import bacc
from concourse.bass_utils import run_bass_kernel_spmd
from concourse.masks import make_identity

HIDDEN = 2048
N_HEADS = 32
N_KV_HEADS = 8
HEAD_DIM = 64
B_FULL, T_FULL = 2, 2048

NCORES = 8
NTP = 4                       # tensor-parallel ranks per batch group
NHL = N_HEADS // NTP          # 8 local q heads
NKVL = N_KV_HEADS // NTP      # 2 local kv heads
QF = NHL * HEAD_DIM           # 512 local q features
KF = NKVL * HEAD_DIM          # 128 local kv features
COLS = HIDDEN // NTP          # 512 output columns per rank
TCP = 256                     # projection t-chunk width
TCA = 512                     # attention window width
P = 128

F32 = mybir.dt.float32
F32R = mybir.dt.float32r
BF16 = mybir.dt.bfloat16

SCALE = 1.0 / np.sqrt(HEAD_DIM)

LAST_EXEC_NS = None
LAST_RESULTS = None

AG_MODE = 'tp4one'


def build_kernel(T=T_FULL, repeat=1, no_ag=False, ag_mode=AG_MODE):
    """One SPMD program; every core runs the same code on its shard."""
    assert T % TCA == 0
    NW = T // TCA             # attention windows
    KH = HIDDEN // P          # 16 k-tiles over hidden
    NTT = T // P              # tk tiles total
    WTK = TCA // P            # tk tiles per window (4)

    nc = bacc.Bacc("TRN2", debug=False)

    NKW = 2 * KH if ag_mode in ('full8', 'full8s', 'single8') else KH

    xT = nc.dram_tensor("xT", [HIDDEN, T], BF16, kind="ExternalInput")
    wqT = nc.dram_tensor("wqT", [HIDDEN, QF], BF16, kind="ExternalInput")
    wkT = nc.dram_tensor("wkT", [HIDDEN, KF], BF16, kind="ExternalInput")
    wvT = nc.dram_tensor("wvT", [HIDDEN, KF], BF16, kind="ExternalInput")
    woT = nc.dram_tensor("woT", [NKW * P, COLS], BF16, kind="ExternalInput")
    cosT = nc.dram_tensor("cosT", [P, T], BF16, kind="ExternalInput")
    sinTs = nc.dram_tensor("sinTs", [P, T], BF16, kind="ExternalInput")
    swp = nc.dram_tensor("swp", [P, P], BF16, kind="ExternalInput")
    msk = nc.dram_tensor("msk", [P, P], BF16, kind="ExternalInput")
    out = nc.dram_tensor("out", [COLS, T], F32, kind="ExternalOutput")

    n_gather = NCORES if ag_mode in ('full8', 'full8s', 'single8') else NTP
    shared_kw = {"addr_space": "Shared"} if ag_mode == "full8s" else {}
    if ag_mode == 'tp4one':
        GSZ = NW                       # ONE collective per rep
    else:
        GSZ = 2 if NW % 2 == 0 else 1  # windows gathered per collective
    NPAIR = NW // GSZ
    cc_in = [nc.dram_tensor(f"cc_in{i}", [QF, GSZ * TCA], BF16)
             for i in range(NPAIR * repeat)]
    cc_out = [nc.dram_tensor(f"cc_out{i}", [n_gather * QF, GSZ * TCA], BF16,
                             **shared_kw)
              for i in range(NPAIR * repeat)]
    groups = ([[0, 1, 2, 3, 4, 5, 6, 7]] if n_gather == 8
              else [[0, 1, 2, 3], [4, 5, 6, 7]])

    with tile.TileContext(nc) as tc, ExitStack() as est:
        consts = est.enter_context(tc.tile_pool(name="consts", bufs=1))
        kpool = est.enter_context(tc.tile_pool(name="kpool", bufs=1))
        xcpool = est.enter_context(tc.tile_pool(name="xcpool", bufs=4))
        stream = est.enter_context(tc.tile_pool(name="stream", bufs=3))
        qrpool = est.enter_context(tc.tile_pool(name="qrpool", bufs=9))
        ppool = est.enter_context(tc.tile_pool(name="ppool", bufs=3))
        atpool = est.enter_context(tc.tile_pool(name="atpool", bufs=2))
        agpool = est.enter_context(tc.tile_pool(name="agpool", bufs=1 if NKW == 32 else 2))
        small = est.enter_context(tc.tile_pool(name="small", bufs=2))
        # PSUM budget: 8 banks of [128 x 2KB].
        #   ps_s:  2 x [128,1024]f32 = 4 banks (pair scores, double-buffered)
        #   ps_pv: 2 x [65,512]f32   = 2 banks (even+odd head accumulators)
        #   ps_ab: 2 x [128,<=512]   = 2 banks (proj / wo / swap / rec shared)
        ps_s = est.enter_context(tc.tile_pool(name="ps_s", bufs=2, space="PSUM"))
        ps_pv = est.enter_context(tc.tile_pool(name="ps_pv", bufs=2, space="PSUM"))
        ps_ab = est.enter_context(tc.tile_pool(name="ps_ab", bufs=2, space="PSUM"))

        # ---- constants (DMA order matters for startup: weights first, then
        # rope tables, mask, wo) ----
        swp_sb = consts.tile([P, P], BF16)
        wq_sb = consts.tile([P, KH, QF], BF16)
        wk_sb = consts.tile([P, KH, KF], BF16)
        wv_sb = consts.tile([P, KH, KF], BF16)
        wo_sb = consts.tile([P, NKW, COLS], BF16)
        cos_sb = consts.tile([P, 2, TCA], BF16)
        sin_sb = consts.tile([P, 2, TCA], BF16)
        msk_sb = consts.tile([P, P], BF16)
        id_sb = consts.tile([P, P], BF16)
        id_f32 = consts.tile([P, P], F32)
        ones_sb = consts.tile([1, HEAD_DIM], F32R)
        ones_f32 = consts.tile([P, 1], F32)
        ones_row_f32 = consts.tile([1, HEAD_DIM], F32)

        xv = xT[:, :].rearrange("(t p) n -> p t n", p=P)
        nc.sync.dma_start(out=swp_sb, in_=swp[:, :])
        wqv = wqT[:, :].rearrange("(t p) f -> p t f", p=P)
        wkv = wkT[:, :].rearrange("(t p) f -> p t f", p=P)
        wvv = wvT[:, :].rearrange("(t p) f -> p t f", p=P)
        wov = woT[:, :].rearrange("(t p) f -> p t f", p=P)
        xc0 = xcpool.tile([P, KH, TCP], BF16, tag="xc")
        nc.sync.dma_start(out=xc0, in_=xv[:, :, 0:TCP])
        nc.sync.dma_start(out=wq_sb, in_=wqv[:, :, :])
        nc.sync.dma_start(out=wk_sb, in_=wkv[:, :, :])
        nc.sync.dma_start(out=wv_sb, in_=wvv[:, :, :])
        sl = slice(0, TCA)
        nc.sync.dma_start(out=cos_sb[:, 0, :], in_=cosT[:, sl])
        nc.sync.dma_start(out=sin_sb[:, 0, :], in_=sinTs[:, sl])
        nc.sync.dma_start(out=msk_sb, in_=msk[:, :])

        make_identity(nc, id_f32)
        nc.vector.tensor_copy(id_sb, id_f32)
        nc.vector.memset(ones_f32, 1.0)
        nc.vector.memset(ones_row_f32, 1.0)
        nc.vector.tensor_copy(ones_sb, ones_row_f32)

        # ---- persistent K / V accumulators ----
        KA = kpool.tile([P, T], BF16, tag="KA")   # [g0; g0] roped K^T
        KB = kpool.tile([P, T], BF16, tag="KB")   # [g1; g1]
        # V natural layout per tk-tile: cols = [V_g0 (64) | 1 | V_g1 (64) | 1]
        vaug = kpool.tile([P, NTT, 2 * HEAD_DIM + 2], BF16, tag="vaug")
        for t in range(NTT):
            nc.vector.tensor_copy(vaug[:, t, HEAD_DIM:HEAD_DIM + 1], ones_f32)
            nc.vector.tensor_copy(vaug[:, t, 2 * HEAD_DIM + 1:2 * HEAD_DIM + 2],
                                  ones_f32)

        def rope(raw_sb, cs, ss, out_ap):
            """out = raw*cos + swap(raw)*sin_signed  (all [P, TCP])."""
            sw_ps = ps_ab.tile([P, TCP], F32, tag="ab")
            nc.tensor.matmul(sw_ps, lhsT=swp_sb, rhs=raw_sb, start=True, stop=True)
            m2 = stream.tile([P, TCP], BF16, tag="tmp")
            nc.vector.tensor_tensor(out=m2, in0=sw_ps, in1=ss, op=mybir.AluOpType.mult)
            nc.vector.tensor_tensor(out=out_ap, in0=raw_sb, in1=cs,
                                    op=mybir.AluOpType.mult)
            nc.vector.tensor_tensor(out=out_ap, in0=out_ap, in1=m2,
                                    op=mybir.AluOpType.add)

        def proj_pieces(c, qrope, half, xc=None):
            """Return filler pieces (closures) for projections of t-chunk c."""
            csl = slice(c * TCP, (c + 1) * TCP)
            hsl = slice(half * TCP, (half + 1) * TCP)
            slot = (c // 2) % 2
            lsl_c = slice((c % 2) * TCP, (c % 2 + 1) * TCP)
            cs = cos_sb[:, slot, lsl_c]
            ss = sin_sb[:, slot, lsl_c]
            if xc is None:
                xc = xcpool.tile([P, KH, TCP], BF16, tag="xc")
                nc.sync.dma_start(out=xc, in_=xv[:, :, csl])

            def q_piece(m):
                def go():
                    q_ps = ps_ab.tile([P, TCP], F32, tag="ab")
                    for k in range(KH):
                        nc.tensor.matmul(q_ps, lhsT=wq_sb[:, k, m * P:(m + 1) * P],
                                         rhs=xc[:, k, :], start=(k == 0),
                                         stop=(k == KH - 1))
                    raw = stream.tile([P, TCP], BF16, tag="raw")
                    nc.vector.tensor_copy(raw, q_ps)
                    rope(raw, cs, ss, qrope[m][:, hsl])
                return go

            def k_piece():
                k_ps = ps_ab.tile([P, TCP], F32, tag="ab")
                for k in range(KH):
                    nc.tensor.matmul(k_ps, lhsT=wk_sb[:, k, :], rhs=xc[:, k, :],
                                     start=(k == 0), stop=(k == KH - 1))
                kraw = stream.tile([P, TCP], BF16, tag="raw")
                nc.vector.tensor_copy(kraw, k_ps)
                krope = stream.tile([P, TCP], BF16, tag="raw")
                rope(kraw, cs, ss, krope)
                nc.vector.tensor_copy(KA[0:64, csl], krope[0:64, :])
                nc.vector.tensor_copy(KA[64:128, csl], krope[0:64, :])
                nc.vector.tensor_copy(KB[0:64, csl], krope[64:128, :])
                nc.vector.tensor_copy(KB[64:128, csl], krope[64:128, :])

            def v_piece():
                v_ps = ps_ab.tile([P, TCP], F32, tag="ab")
                for k in range(KH):
                    nc.tensor.matmul(v_ps, lhsT=wv_sb[:, k, :], rhs=xc[:, k, :],
                                     start=(k == 0), stop=(k == KH - 1))
                vt = stream.tile([P, TCP], BF16, tag="raw")
                nc.vector.tensor_copy(vt, v_ps)
                for tt in range(TCP // P):
                    tp_ps = ps_ab.tile([P, P], BF16, tag="ab")
                    nc.tensor.transpose(tp_ps, vt[:, tt * P:(tt + 1) * P], id_sb)
                    tkt = c * (TCP // P) + tt
                    nc.vector.tensor_copy(vaug[:, tkt, 0:HEAD_DIM],
                                          tp_ps[:, 0:HEAD_DIM])
                    nc.vector.tensor_copy(vaug[:, tkt, HEAD_DIM + 1:2 * HEAD_DIM + 1],
                                          tp_ps[:, HEAD_DIM:2 * HEAD_DIM])

            return ([q_piece(m) for m in range(4)] + [k_piece, v_piece])

        def wo_pieces(pi, w0):
            """Filler pieces for the output projection of gathered pair pi."""
            ccv = cc_out[pi][:, :].rearrange("(t p) n -> p t n", p=P)
            pieces = []
            for sw in range(GSZ):
                ssl = slice(sw * TCA, (sw + 1) * TCA)
                osl = slice((w0 + sw) * TCA, (w0 + sw + 1) * TCA)
                ag = [None]

                def load_ag(ssl=ssl, ag=ag):
                    ag_t = agpool.tile([P, NKW, TCA], BF16, tag="ag")
                    nc.sync.dma_start(out=ag_t, in_=ccv[:, :, ssl])
                    ag[0] = ag_t

                pieces.append(load_ag)
                y4 = small.tile([P, 4, TCA], F32, tag="y4")
                for m in range(4):
                    def mm(m=m, osl=osl, ag=ag, y4=y4):
                        y_ps = ps_ab.tile([P, TCA], F32, tag="ab")
                        for k in range(NKW):
                            nc.tensor.matmul(y_ps,
                                             lhsT=wo_sb[:, k, m * P:(m + 1) * P],
                                             rhs=ag[0][:, k, :], start=(k == 0),
                                             stop=(k == NKW - 1))
                        nc.vector.tensor_copy(y4[:, m, :], y_ps)
                        if m == 3:
                            ov = out[:, :].rearrange("(m p) t -> p m t", p=P)
                            nc.sync.dma_start(out=ov[:, :, osl], in_=y4)
                    pieces.append(mm)
            return pieces

        fillers = []              # queue of (tag, fn) pending filler pieces

        def emit_filler(n=1):
            for _ in range(n):
                if fillers:
                    fillers.pop(0)[1]()

        def flush_tag(tag):
            while any(t == tag for t, _ in fillers):
                fillers.pop(0)[1]()

        for rep in range(repeat):
            for w in range(NW):
                wsl = slice(w * TCA, (w + 1) * TCA)
                first = (rep == 0 and w == 0)
                if first:
                    qrope = []
                    for _ in range(4):
                        qr_t = qrpool.tile([P, TCA], BF16, tag="qrope")
                        qrope.append(qr_t)
                    for p_ in proj_pieces(2 * w, qrope, 0, xc=xc0):
                        p_()
                    for p_ in proj_pieces(2 * w + 1, qrope, 1):
                        p_()
                    qrope_next = None
                else:
                    qrope = qrope_next

                # queue next window's projections as fillers
                if not (w + 1 == NW and rep + 1 == repeat):
                    nw_ = (w + 1) % NW
                    qrope_next = []
                    for _ in range(4):
                        qr_t = qrpool.tile([P, TCA], BF16, tag="qrope")
                        qrope_next.append(qr_t)
                    nsl = slice(nw_ * TCA, (nw_ + 1) * TCA)
                    nslot = (w + 1) % 2
                    nc.sync.dma_start(out=cos_sb[:, nslot, :], in_=cosT[:, nsl])
                    nc.sync.dma_start(out=sin_sb[:, nslot, :], in_=sinTs[:, nsl])
                    gw1 = rep * NW + w + 1
                    fillers.extend((("proj", gw1), p_)
                                   for p_ in proj_pieces(2 * nw_, qrope_next, 0))
                    fillers.extend((("proj", gw1), p_)
                                   for p_ in proj_pieces(2 * nw_ + 1, qrope_next, 1))

                # all proj pieces for THIS window must be emitted before its
                # attention reads qrope/KA/KB/vaug (they may still be queued)
                flush_tag(("proj", rep * NW + w))

                # ---- attention window: 4 head-pairs ----
                n_tk = (w + 1) * WTK
                at4 = atpool.tile([P, 4, TCA], BF16, tag="attnT")
                at_tiles = [at4[:, m_, :] for m_ in range(4)]
                for m in range(4):
                    g = m // 2
                    ksrc = KA if g == 0 else KB
                    qt = qrope[m]
                    vsl = slice(g * (HEAD_DIM + 1), (g + 1) * (HEAD_DIM + 1))

                    pv_e = ps_pv.tile([HEAD_DIM + 1, TCA], F32, tag="pv")
                    pv_o = ps_pv.tile([HEAD_DIM + 1, TCA], F32, tag="pv")
                    for i in range(n_tk):
                        o = i - w * WTK
                        lo = max(o, 0) * P
                        s_ps = ps_s.tile([P, 2 * TCA], F32, tag="s")
                        nc.tensor.matmul(
                            s_ps[:, lo:TCA],
                            lhsT=ksrc[0:HEAD_DIM, i * P:(i + 1) * P],
                            rhs=qt[0:HEAD_DIM, lo:],
                            start=True, stop=True)
                        nc.tensor.matmul(
                            s_ps[:, TCA + lo:],
                            lhsT=ksrc[HEAD_DIM:P, i * P:(i + 1) * P],
                            rhs=qt[HEAD_DIM:P, lo:],
                            start=True, stop=True)
                        p_sb = ppool.tile([P, 2 * TCA], BF16, tag="p")
                        sv = s_ps[:, :].rearrange("p (b c) -> p b c", b=2)
                        pvw = p_sb[:, :].rearrange("p (b c) -> p b c", b=2)
                        nc.scalar.activation(out=pvw[:, :, lo:], in_=sv[:, :, lo:],
                                             func=mybir.ActivationFunctionType.Exp,
                                             scale=float(SCALE))
                        if o >= 0:
                            for b_ in range(2):
                                nc.vector.tensor_tensor(
                                    out=p_sb[:, b_ * TCA + lo:b_ * TCA + lo + P],
                                    in0=p_sb[:, b_ * TCA + lo:b_ * TCA + lo + P],
                                    in1=msk_sb,
                                    op=mybir.AluOpType.mult)
                        nc.tensor.matmul(pv_e[:, lo:], lhsT=vaug[:, i, vsl],
                                         rhs=p_sb[:, lo:TCA],
                                         start=(i == 0), stop=(i == n_tk - 1))
                        nc.tensor.matmul(pv_o[:, lo:], lhsT=vaug[:, i, vsl],
                                         rhs=p_sb[:, TCA + lo:],
                                         start=(i == 0), stop=(i == n_tk - 1))
                        if i % 3 == 2 and i + 1 < n_tk:
                            emit_filler(1)

                    for par, pv_ps in ((0, pv_e), (1, pv_o)):
                        base = par * HEAD_DIM
                        rec = small.tile([1, TCA], F32R, tag="recip")
                        with nc.allow_low_precision(reason="f32r softmax denom"):
                            nc.vector.reciprocal(rec,
                                                 pv_ps[HEAD_DIM:HEAD_DIM + 1, :])
                        rep_ps = ps_ab.tile([HEAD_DIM, TCA], F32, tag="ab")
                        nc.tensor.matmul(rep_ps, lhsT=ones_sb, rhs=rec,
                                         start=True, stop=True)
                        rep_sb = small.tile([HEAD_DIM, TCA], F32, tag="rep")
                        nc.scalar.activation(out=rep_sb, in_=rep_ps,
                                             func=mybir.ActivationFunctionType.Copy)
                        nc.vector.tensor_tensor(
                            out=at_tiles[m][base:base + HEAD_DIM, :],
                            in0=pv_ps[0:HEAD_DIM, :], in1=rep_sb,
                            op=mybir.AluOpType.mult)
                    emit_filler(2 if w >= 2 else 1)

                # ---- collective of attn^T window across ranks ----
                pi = rep * NPAIR + w // GSZ
                psl = slice((w % GSZ) * TCA, (w % GSZ + 1) * TCA)
                civ = cc_in[pi][:, :].rearrange("(m p) t -> p m t", p=P)
                nc.sync.dma_start(out=civ[:, :, psl], in_=at4)
                if w % GSZ == GSZ - 1:
                    if not no_ag:
                        nc.gpsimd.collective_compute(
                            "AllGather", mybir.AluOpType.bypass,
                            replica_groups=groups,
                            ins=[cc_in[pi][:, :]],
                            outs=[cc_out[pi][:, :]],
                        )
                    fillers.extend((("wo", pi), p_)
                                   for p_ in wo_pieces(pi, w - GSZ + 1))

                if rep == 0 and w == 0:
                    for k in range(NKW):
                        nc.sync.dma_start(out=wo_sb[:, k, :], in_=wov[:, k, :])

        while fillers:
            emit_filler(1)

    nc.compile()
    return nc


_NC_CACHE = {}


def _get_nc(T):
    if T not in _NC_CACHE:
        _NC_CACHE[T] = build_kernel(T, ag_mode=AG_MODE)
    return _NC_CACHE[T]


def _perm64():
    """Per-head permutation: interleaved (even,odd) -> [r(32) | i(32)]."""
    p = np.empty(HEAD_DIM, dtype=np.int64)
    p[:32] = np.arange(0, HEAD_DIM, 2)
    p[32:] = np.arange(1, HEAD_DIM, 2)
    return p


def make_inputs(x, freqs_cis, wq, wk, wv, wo, T, ag_mode=None):
    """Build the 8 per-core input maps (host-side sharding + layout prep)."""
    ag_mode = ag_mode or AG_MODE
    perm = _perm64()
    f32 = np.float32

    cos = np.asarray(freqs_cis[:T, :, 0], dtype=f32)   # [T, 32]
    sin = np.asarray(freqs_cis[:T, :, 1], dtype=f32)
    cosT = np.tile(cos.T, (4, 1)).astype(f32)                        # [128, T]
    sinTs = np.tile(np.vstack([-sin.T, sin.T]), (2, 1)).astype(f32)  # [128, T]

    J = np.zeros((HEAD_DIM, HEAD_DIM), dtype=f32)
    J[np.arange(32), np.arange(32) + 32] = 1.0
    J[np.arange(32) + 32, np.arange(32)] = 1.0
    swp = np.zeros((P, P), dtype=f32)
    swp[:HEAD_DIM, :HEAD_DIM] = J
    swp[HEAD_DIM:, HEAD_DIM:] = J

    # single causal triangle mask [128, 128]: msk[p, q] = (q >= p)
    q_idx = np.arange(P)
    p_idx = np.arange(P)[:, None]
    msk = (q_idx[None, :] >= p_idx).astype(f32)

    def permute_heads(w, n_heads):
        wh = np.asarray(w, f32).reshape(n_heads, HEAD_DIM, HIDDEN)
        return wh[:, perm, :].reshape(n_heads * HEAD_DIM, HIDDEN)

    wq_p = permute_heads(wq, N_HEADS)
    wk_p = permute_heads(wk, N_KV_HEADS)
    wv_n = np.asarray(wv, f32)
    wo_n = np.asarray(wo, f32)

    bf16 = ml_dtypes.bfloat16
    in_maps = []
    for core in range(NCORES):
        b, j = divmod(core, NTP)
        xTc = np.ascontiguousarray(np.asarray(x[b, :T], f32).T.astype(bf16))
        wqTc = np.ascontiguousarray(wq_p[j * QF:(j + 1) * QF].T.astype(bf16))
        wkTc = np.ascontiguousarray(wk_p[j * KF:(j + 1) * KF].T.astype(bf16))
        wvTc = np.ascontiguousarray(wv_n[j * KF:(j + 1) * KF].T.astype(bf16))
        wo_own = wo_n[j * COLS:(j + 1) * COLS].T.astype(bf16)
        if ag_mode in ('full8', 'full8s', 'single8'):
            woTc = np.zeros((2 * HIDDEN, COLS), dtype=bf16)
            woTc[b * HIDDEN:(b + 1) * HIDDEN] = wo_own  # own-batch rows only
        else:
            woTc = np.ascontiguousarray(wo_own)
        in_maps.append({
            "xT": xTc, "wqT": wqTc, "wkT": wkTc, "wvT": wvTc, "woT": woTc,
            "cosT": cosT.astype(bf16), "sinTs": sinTs.astype(bf16),
            "swp": swp.astype(bf16), "msk": msk.astype(bf16),
        })
    return in_maps


def kernel(x, freqs_cis, wq, wk, wv, wo):
    global LAST_EXEC_NS, LAST_RESULTS
    T = x.shape[1]
    nc = _get_nc(T)
    in_maps = make_inputs(x, freqs_cis, wq, wk, wv, wo, T)
    trace = bool(int(os.environ.get("KERNEL_TRACE", "0")))
    res = run_bass_kernel_spmd(nc, in_maps, core_ids=list(range(NCORES)),
                               trace=trace)
    LAST_EXEC_NS = res.exec_time_ns
    LAST_RESULTS = res
    out = np.empty((B_FULL, T, HIDDEN), dtype=np.float32)
    for core in range(NCORES):
        b, j = divmod(core, NTP)
        out[b, :, j * COLS:(j + 1) * COLS] = res.results[core]["out"].T
    return out


# revision 13
# speedup vs baseline: 1.6404x; 1.1404x over previous
"""GQA attention (dense_transformer) on 8 TRN2 NeuronCores.

Sharding: core c = b*4 + j  (b = batch 0..1, j = tensor-parallel rank 0..3).
Each core computes q-heads 8j..8j+7 (kv heads 2j, 2j+1) for batch b, then a
collective of attn^T, then its shard of the output projection.

v2 structure vs v1:
  - attention runs per head-PAIR: scores for both heads land in one
    [128, 2*TCA] PSUM tile -> ONE exp activation per k-tile (halves the
    fixed per-instruction ACT overhead that made the attention chain
    ACT-latency-bound).
  - projections of window w+1 and the wo matmuls of the gathered window
    pair are emitted as "filler" pieces interleaved into the attention
    loop, so PE (in-order) has work while exp stalls the score->PV chain.
  - collective mode selectable; default tp4one: a SINGLE 4-rank
    AllGather per rep (collectives here cost ~200us fixed each and do
    not pipeline, so one big gather beats two half-size ones).
"""
import os
import sys

sys.path.insert(0, "/opt/trn_rl_repo")

from contextlib import ExitStack

import numpy as np
import ml_dtypes

import concourse.bass as bass
import concourse.mybir as mybir
import concourse.tile as tile
from concourse import bacc
from concourse.bass_utils import run_bass_kernel_spmd
from concourse.masks import make_identity

HIDDEN = 2048
N_HEADS = 32
N_KV_HEADS = 8
HEAD_DIM = 64
B_FULL, T_FULL = 2, 2048

NCORES = 8
NTP = 4                       # tensor-parallel ranks per batch group
NHL = N_HEADS // NTP          # 8 local q heads
NKVL = N_KV_HEADS // NTP      # 2 local kv heads
QF = NHL * HEAD_DIM           # 512 local q features
KF = NKVL * HEAD_DIM          # 128 local kv features
COLS = HIDDEN // NTP          # 512 output columns per rank
TCP = 256                     # projection t-chunk width
TCA = 512                     # attention window width
P = 128

F32 = mybir.dt.float32
F32R = mybir.dt.float32r
BF16 = mybir.dt.bfloat16

SCALE = 1.0 / np.sqrt(HEAD_DIM)

LAST_EXEC_NS = None
LAST_RESULTS = None

AG_MODE = 'tp4one'


def build_kernel(T=T_FULL, repeat=1, no_ag=False, ag_mode=AG_MODE):
    """One SPMD program; every core runs the same code on its shard."""
    assert T % TCA == 0
    NW = T // TCA             # attention windows
    KH = HIDDEN // P          # 16 k-tiles over hidden
    NTT = T // P              # tk tiles total
    WTK = TCA // P            # tk tiles per window (4)

    nc = bacc.Bacc("TRN2", debug=False)

    NKW = 2 * KH if ag_mode in ('full8', 'full8s', 'single8') else KH

    xT = nc.dram_tensor("xT", [HIDDEN, T], BF16, kind="ExternalInput")
    wqT = nc.dram_tensor("wqT", [HIDDEN, QF], BF16, kind="ExternalInput")
    wkT = nc.dram_tensor("wkT", [HIDDEN, KF], BF16, kind="ExternalInput")
    wvT = nc.dram_tensor("wvT", [HIDDEN, KF], BF16, kind="ExternalInput")
    woT = nc.dram_tensor("woT", [NKW * P, COLS], BF16, kind="ExternalInput")
    cosT = nc.dram_tensor("cosT", [P, T], BF16, kind="ExternalInput")
    sinTs = nc.dram_tensor("sinTs", [P, T], BF16, kind="ExternalInput")
    swp = nc.dram_tensor("swp", [P, P], BF16, kind="ExternalInput")
    msk = nc.dram_tensor("msk", [P, P], BF16, kind="ExternalInput")
    out = nc.dram_tensor("out", [COLS, T], F32, kind="ExternalOutput")

    n_gather = NCORES if ag_mode in ('full8', 'full8s', 'single8') else NTP
    shared_kw = {"addr_space": "Shared"} if ag_mode == "full8s" else {}
    if ag_mode == 'tp4one':
        GSZ = NW                       # ONE collective per rep
    else:
        GSZ = 2 if NW % 2 == 0 else 1  # windows gathered per collective
    NPAIR = NW // GSZ
    cc_in = [nc.dram_tensor(f"cc_in{i}", [QF, GSZ * TCA], BF16)
             for i in range(NPAIR * repeat)]
    cc_out = [nc.dram_tensor(f"cc_out{i}", [n_gather * QF, GSZ * TCA], BF16,
                             **shared_kw)
              for i in range(NPAIR * repeat)]
    groups = ([[0, 1, 2, 3, 4, 5, 6, 7]] if n_gather == 8
              else [[0, 1, 2, 3], [4, 5, 6, 7]])

    with tile.TileContext(nc) as tc, ExitStack() as est:
        consts = est.enter_context(tc.tile_pool(name="consts", bufs=1))
        kpool = est.enter_context(tc.tile_pool(name="kpool", bufs=1))
        xcpool = est.enter_context(tc.tile_pool(name="xcpool", bufs=4))
        stream = est.enter_context(tc.tile_pool(name="stream", bufs=3))
        qrpool = est.enter_context(tc.tile_pool(name="qrpool", bufs=9))
        ppool = est.enter_context(tc.tile_pool(name="ppool", bufs=3))
        atpool = est.enter_context(tc.tile_pool(name="atpool", bufs=2))
        agpool = est.enter_context(tc.tile_pool(name="agpool", bufs=1 if NKW == 32 else 2))
        small = est.enter_context(tc.tile_pool(name="small", bufs=2))
        # PSUM budget: 8 banks of [128 x 2KB].
        #   ps_s:  2 x [128,1024]f32 = 4 banks (pair scores, double-buffered)
        #   ps_pv: 2 x [65,512]f32   = 2 banks (even+odd head accumulators)
        #   ps_ab: 2 x [128,<=512]   = 2 banks (proj / wo / swap / rec shared)
        ps_s = est.enter_context(tc.tile_pool(name="ps_s", bufs=2, space="PSUM"))
        ps_pv = est.enter_context(tc.tile_pool(name="ps_pv", bufs=2, space="PSUM"))
        ps_ab = est.enter_context(tc.tile_pool(name="ps_ab", bufs=2, space="PSUM"))

        # ---- constants (DMA order matters for startup: weights first, then
        # rope tables, mask, wo) ----
        swp_sb = consts.tile([P, P], BF16)
        wq_sb = consts.tile([P, KH, QF], BF16)
        wk_sb = consts.tile([P, KH, KF], BF16)
        wv_sb = consts.tile([P, KH, KF], BF16)
        wo_sb = consts.tile([P, NKW, COLS], BF16)
        cos_sb = consts.tile([P, 2, TCA], BF16)
        sin_sb = consts.tile([P, 2, TCA], BF16)
        msk_sb = consts.tile([P, P], BF16)
        id_sb = consts.tile([P, P], BF16)
        id_f32 = consts.tile([P, P], F32)
        ones_sb = consts.tile([1, HEAD_DIM], F32R)
        ones_f32 = consts.tile([P, 1], F32)
        ones_row_f32 = consts.tile([1, HEAD_DIM], F32)

        xv = xT[:, :].rearrange("(t p) n -> p t n", p=P)
        nc.sync.dma_start(out=swp_sb, in_=swp[:, :])
        wqv = wqT[:, :].rearrange("(t p) f -> p t f", p=P)
        wkv = wkT[:, :].rearrange("(t p) f -> p t f", p=P)
        wvv = wvT[:, :].rearrange("(t p) f -> p t f", p=P)
        wov = woT[:, :].rearrange("(t p) f -> p t f", p=P)
        xc0 = xcpool.tile([P, KH, TCP], BF16, tag="xc")
        nc.sync.dma_start(out=xc0, in_=xv[:, :, 0:TCP])
        for kq in range(4):
            nc.sync.dma_start(out=wq_sb[:, 4 * kq:4 * kq + 4, :],
                              in_=wqv[:, 4 * kq:4 * kq + 4, :])
        nc.sync.dma_start(out=wk_sb, in_=wkv[:, :, :])
        nc.sync.dma_start(out=wv_sb, in_=wvv[:, :, :])
        sl = slice(0, TCA)
        nc.sync.dma_start(out=cos_sb[:, 0, :], in_=cosT[:, sl])
        nc.sync.dma_start(out=sin_sb[:, 0, :], in_=sinTs[:, sl])
        nc.sync.dma_start(out=msk_sb, in_=msk[:, :])

        make_identity(nc, id_f32)
        nc.vector.tensor_copy(id_sb, id_f32)
        nc.vector.memset(ones_f32, 1.0)
        nc.vector.memset(ones_row_f32, 1.0)
        nc.vector.tensor_copy(ones_sb, ones_row_f32)

        # ---- persistent K / V accumulators ----
        KA = kpool.tile([P, T], BF16, tag="KA")   # [g0; g0] roped K^T
        KB = kpool.tile([P, T], BF16, tag="KB")   # [g1; g1]
        # V natural layout per tk-tile: cols = [V_g0 (64) | 1 | V_g1 (64) | 1]
        vaug = kpool.tile([P, NTT, 2 * HEAD_DIM + 2], BF16, tag="vaug")
        for t in range(NTT):
            nc.vector.tensor_copy(vaug[:, t, HEAD_DIM:HEAD_DIM + 1], ones_f32)
            nc.vector.tensor_copy(vaug[:, t, 2 * HEAD_DIM + 1:2 * HEAD_DIM + 2],
                                  ones_f32)

        def rope(raw_sb, cs, ss, out_ap):
            """out = raw*cos + swap(raw)*sin_signed  (all [P, TCP])."""
            sw_ps = ps_ab.tile([P, TCP], F32, tag="ab")
            nc.tensor.matmul(sw_ps, lhsT=swp_sb, rhs=raw_sb, start=True, stop=True)
            m2 = stream.tile([P, TCP], BF16, tag="tmp")
            nc.vector.tensor_tensor(out=m2, in0=sw_ps, in1=ss, op=mybir.AluOpType.mult)
            nc.vector.tensor_tensor(out=out_ap, in0=raw_sb, in1=cs,
                                    op=mybir.AluOpType.mult)
            nc.vector.tensor_tensor(out=out_ap, in0=out_ap, in1=m2,
                                    op=mybir.AluOpType.add)

        def proj_pieces(c, qrope, half, xc=None):
            """Return filler pieces (closures) for projections of t-chunk c."""
            csl = slice(c * TCP, (c + 1) * TCP)
            hsl = slice(half * TCP, (half + 1) * TCP)
            slot = (c // 2) % 2
            lsl_c = slice((c % 2) * TCP, (c % 2 + 1) * TCP)
            cs = cos_sb[:, slot, lsl_c]
            ss = sin_sb[:, slot, lsl_c]
            if xc is None:
                xc = xcpool.tile([P, KH, TCP], BF16, tag="xc")
                nc.sync.dma_start(out=xc, in_=xv[:, :, csl])

            def q_piece(m):
                def go():
                    q_ps = ps_ab.tile([P, TCP], F32, tag="ab")
                    for k in range(KH):
                        nc.tensor.matmul(q_ps, lhsT=wq_sb[:, k, m * P:(m + 1) * P],
                                         rhs=xc[:, k, :], start=(k == 0),
                                         stop=(k == KH - 1))
                    raw = stream.tile([P, TCP], BF16, tag="raw")
                    nc.vector.tensor_copy(raw, q_ps)
                    rope(raw, cs, ss, qrope[m][:, hsl])
                return go

            def k_piece():
                k_ps = ps_ab.tile([P, TCP], F32, tag="ab")
                for k in range(KH):
                    nc.tensor.matmul(k_ps, lhsT=wk_sb[:, k, :], rhs=xc[:, k, :],
                                     start=(k == 0), stop=(k == KH - 1))
                kraw = stream.tile([P, TCP], BF16, tag="raw")
                nc.vector.tensor_copy(kraw, k_ps)
                krope = stream.tile([P, TCP], BF16, tag="raw")
                rope(kraw, cs, ss, krope)
                nc.vector.tensor_copy(KA[0:64, csl], krope[0:64, :])
                nc.vector.tensor_copy(KA[64:128, csl], krope[0:64, :])
                nc.vector.tensor_copy(KB[0:64, csl], krope[64:128, :])
                nc.vector.tensor_copy(KB[64:128, csl], krope[64:128, :])

            def v_piece():
                v_ps = ps_ab.tile([P, TCP], F32, tag="ab")
                for k in range(KH):
                    nc.tensor.matmul(v_ps, lhsT=wv_sb[:, k, :], rhs=xc[:, k, :],
                                     start=(k == 0), stop=(k == KH - 1))
                vt = stream.tile([P, TCP], BF16, tag="raw")
                nc.vector.tensor_copy(vt, v_ps)
                for tt in range(TCP // P):
                    tp_ps = ps_ab.tile([P, P], BF16, tag="ab")
                    nc.tensor.transpose(tp_ps, vt[:, tt * P:(tt + 1) * P], id_sb)
                    tkt = c * (TCP // P) + tt
                    nc.vector.tensor_copy(vaug[:, tkt, 0:HEAD_DIM],
                                          tp_ps[:, 0:HEAD_DIM])
                    nc.vector.tensor_copy(vaug[:, tkt, HEAD_DIM + 1:2 * HEAD_DIM + 1],
                                          tp_ps[:, HEAD_DIM:2 * HEAD_DIM])

            return ([q_piece(m) for m in range(4)] + [k_piece, v_piece])

        def wo_pieces(pi, w0):
            """Filler pieces for the output projection of gathered pair pi."""
            ccv = cc_out[pi][:, :].rearrange("(t p) n -> p t n", p=P)
            pieces = []
            for sw in range(GSZ):
                ssl = slice(sw * TCA, (sw + 1) * TCA)
                osl = slice((w0 + sw) * TCA, (w0 + sw + 1) * TCA)
                ag = [None]

                def load_ag(ssl=ssl, ag=ag):
                    ag_t = agpool.tile([P, NKW, TCA], BF16, tag="ag")
                    nc.sync.dma_start(out=ag_t, in_=ccv[:, :, ssl])
                    ag[0] = ag_t

                pieces.append(load_ag)
                y4 = small.tile([P, 4, TCA], F32, tag="y4")
                for m in range(4):
                    def mm(m=m, osl=osl, ag=ag, y4=y4):
                        y_ps = ps_ab.tile([P, TCA], F32, tag="ab")
                        for k in range(NKW):
                            nc.tensor.matmul(y_ps,
                                             lhsT=wo_sb[:, k, m * P:(m + 1) * P],
                                             rhs=ag[0][:, k, :], start=(k == 0),
                                             stop=(k == NKW - 1))
                        nc.vector.tensor_copy(y4[:, m, :], y_ps)
                        if m == 3:
                            ov = out[:, :].rearrange("(m p) t -> p m t", p=P)
                            nc.sync.dma_start(out=ov[:, :, osl], in_=y4)
                    pieces.append(mm)
            return pieces

        fillers = []              # queue of (tag, fn) pending filler pieces

        def emit_filler(n=1):
            for _ in range(n):
                if fillers:
                    fillers.pop(0)[1]()

        def flush_tag(tag):
            while any(t == tag for t, _ in fillers):
                fillers.pop(0)[1]()

        for rep in range(repeat):
            for w in range(NW):
                wsl = slice(w * TCA, (w + 1) * TCA)
                first = (rep == 0 and w == 0)
                if first:
                    qrope = []
                    for _ in range(4):
                        qr_t = qrpool.tile([P, TCA], BF16, tag="qrope")
                        qrope.append(qr_t)
                    for p_ in proj_pieces(2 * w, qrope, 0, xc=xc0):
                        p_()
                    for p_ in proj_pieces(2 * w + 1, qrope, 1):
                        p_()
                    qrope_next = None
                else:
                    qrope = qrope_next

                # queue next window's projections as fillers
                if not (w + 1 == NW and rep + 1 == repeat):
                    nw_ = (w + 1) % NW
                    qrope_next = []
                    for _ in range(4):
                        qr_t = qrpool.tile([P, TCA], BF16, tag="qrope")
                        qrope_next.append(qr_t)
                    nsl = slice(nw_ * TCA, (nw_ + 1) * TCA)
                    nslot = (w + 1) % 2
                    nc.sync.dma_start(out=cos_sb[:, nslot, :], in_=cosT[:, nsl])
                    nc.sync.dma_start(out=sin_sb[:, nslot, :], in_=sinTs[:, nsl])
                    gw1 = rep * NW + w + 1
                    fillers.extend((("proj", gw1), p_)
                                   for p_ in proj_pieces(2 * nw_, qrope_next, 0))
                    fillers.extend((("proj", gw1), p_)
                                   for p_ in proj_pieces(2 * nw_ + 1, qrope_next, 1))

                # all proj pieces for THIS window must be emitted before its
                # attention reads qrope/KA/KB/vaug (they may still be queued)
                flush_tag(("proj", rep * NW + w))

                # ---- attention window: 4 head-pairs ----
                n_tk = (w + 1) * WTK
                at4 = atpool.tile([P, 4, TCA], BF16, tag="attnT")
                at_tiles = [at4[:, m_, :] for m_ in range(4)]
                for m in range(4):
                    g = m // 2
                    ksrc = KA if g == 0 else KB
                    qt = qrope[m]
                    vsl = slice(g * (HEAD_DIM + 1), (g + 1) * (HEAD_DIM + 1))

                    pv_e = ps_pv.tile([HEAD_DIM + 1, TCA], F32, tag="pv")
                    pv_o = ps_pv.tile([HEAD_DIM + 1, TCA], F32, tag="pv")
                    for i in range(n_tk):
                        o = i - w * WTK
                        lo = max(o, 0) * P
                        s_ps = ps_s.tile([P, 2 * TCA], F32, tag="s")
                        nc.tensor.matmul(
                            s_ps[:, lo:TCA],
                            lhsT=ksrc[0:HEAD_DIM, i * P:(i + 1) * P],
                            rhs=qt[0:HEAD_DIM, lo:],
                            start=True, stop=True)
                        nc.tensor.matmul(
                            s_ps[:, TCA + lo:],
                            lhsT=ksrc[HEAD_DIM:P, i * P:(i + 1) * P],
                            rhs=qt[HEAD_DIM:P, lo:],
                            start=True, stop=True)
                        p_sb = ppool.tile([P, 2 * TCA], BF16, tag="p")
                        sv = s_ps[:, :].rearrange("p (b c) -> p b c", b=2)
                        pvw = p_sb[:, :].rearrange("p (b c) -> p b c", b=2)
                        nc.scalar.activation(out=pvw[:, :, lo:], in_=sv[:, :, lo:],
                                             func=mybir.ActivationFunctionType.Exp,
                                             scale=float(SCALE))
                        if o >= 0:
                            for b_ in range(2):
                                nc.vector.tensor_tensor(
                                    out=p_sb[:, b_ * TCA + lo:b_ * TCA + lo + P],
                                    in0=p_sb[:, b_ * TCA + lo:b_ * TCA + lo + P],
                                    in1=msk_sb,
                                    op=mybir.AluOpType.mult)
                        nc.tensor.matmul(pv_e[:, lo:], lhsT=vaug[:, i, vsl],
                                         rhs=p_sb[:, lo:TCA],
                                         start=(i == 0), stop=(i == n_tk - 1))
                        nc.tensor.matmul(pv_o[:, lo:], lhsT=vaug[:, i, vsl],
                                         rhs=p_sb[:, TCA + lo:],
                                         start=(i == 0), stop=(i == n_tk - 1))
                        if i % 3 == 2 and i + 1 < n_tk:
                            emit_filler(1)

                    for par, pv_ps in ((0, pv_e), (1, pv_o)):
                        base = par * HEAD_DIM
                        rec = small.tile([1, TCA], F32R, tag="recip")
                        with nc.allow_low_precision(reason="f32r softmax denom"):
                            nc.vector.reciprocal(rec,
                                                 pv_ps[HEAD_DIM:HEAD_DIM + 1, :])
                        rep_ps = ps_ab.tile([HEAD_DIM, TCA], F32, tag="ab")
                        nc.tensor.matmul(rep_ps, lhsT=ones_sb, rhs=rec,
                                         start=True, stop=True)
                        rep_sb = small.tile([HEAD_DIM, TCA], F32, tag="rep")
                        nc.scalar.activation(out=rep_sb, in_=rep_ps,
                                             func=mybir.ActivationFunctionType.Copy)
                        nc.vector.tensor_tensor(
                            out=at_tiles[m][base:base + HEAD_DIM, :],
                            in0=pv_ps[0:HEAD_DIM, :], in1=rep_sb,
                            op=mybir.AluOpType.mult)
                    emit_filler(2 if w >= 2 else 1)

                # ---- collective of attn^T window across ranks ----
                pi = rep * NPAIR + w // GSZ
                psl = slice((w % GSZ) * TCA, (w % GSZ + 1) * TCA)
                civ = cc_in[pi][:, :].rearrange("(m p) t -> p m t", p=P)
                nc.sync.dma_start(out=civ[:, :, psl], in_=at4)
                if w % GSZ == GSZ - 1:
                    if not no_ag:
                        nc.gpsimd.collective_compute(
                            "AllGather", mybir.AluOpType.bypass,
                            replica_groups=groups,
                            ins=[cc_in[pi][:, :]],
                            outs=[cc_out[pi][:, :]],
                        )
                    fillers.extend((("wo", pi), p_)
                                   for p_ in wo_pieces(pi, w - GSZ + 1))

                if rep == 0 and w == 0:
                    for k in range(NKW):
                        nc.sync.dma_start(out=wo_sb[:, k, :], in_=wov[:, k, :])

        while fillers:
            emit_filler(1)

    nc.compile()
    return nc


_NC_CACHE = {}


def _get_nc(T):
    if T not in _NC_CACHE:
        _NC_CACHE[T] = build_kernel(T, ag_mode=AG_MODE)
    return _NC_CACHE[T]


def _perm64():
    """Per-head permutation: interleaved (even,odd) -> [r(32) | i(32)]."""
    p = np.empty(HEAD_DIM, dtype=np.int64)
    p[:32] = np.arange(0, HEAD_DIM, 2)
    p[32:] = np.arange(1, HEAD_DIM, 2)
    return p


def make_inputs(x, freqs_cis, wq, wk, wv, wo, T, ag_mode=None):
    """Build the 8 per-core input maps (host-side sharding + layout prep)."""
    ag_mode = ag_mode or AG_MODE
    perm = _perm64()
    f32 = np.float32

    cos = np.asarray(freqs_cis[:T, :, 0], dtype=f32)   # [T, 32]
    sin = np.asarray(freqs_cis[:T, :, 1], dtype=f32)
    cosT = np.tile(cos.T, (4, 1)).astype(f32)                        # [128, T]
    sinTs = np.tile(np.vstack([-sin.T, sin.T]), (2, 1)).astype(f32)  # [128, T]

    J = np.zeros((HEAD_DIM, HEAD_DIM), dtype=f32)
    J[np.arange(32), np.arange(32) + 32] = 1.0
    J[np.arange(32) + 32, np.arange(32)] = 1.0
    swp = np.zeros((P, P), dtype=f32)
    swp[:HEAD_DIM, :HEAD_DIM] = J
    swp[HEAD_DIM:, HEAD_DIM:] = J

    # single causal triangle mask [128, 128]: msk[p, q] = (q >= p)
    q_idx = np.arange(P)
    p_idx = np.arange(P)[:, None]
    msk = (q_idx[None, :] >= p_idx).astype(f32)

    def permute_heads(w, n_heads):
        wh = np.asarray(w, f32).reshape(n_heads, HEAD_DIM, HIDDEN)
        return wh[:, perm, :].reshape(n_heads * HEAD_DIM, HIDDEN)

    wq_p = permute_heads(wq, N_HEADS)
    wk_p = permute_heads(wk, N_KV_HEADS)
    wv_n = np.asarray(wv, f32)
    wo_n = np.asarray(wo, f32)

    bf16 = ml_dtypes.bfloat16
    in_maps = []
    for core in range(NCORES):
        b, j = divmod(core, NTP)
        xTc = np.ascontiguousarray(np.asarray(x[b, :T], f32).T.astype(bf16))
        wqTc = np.ascontiguousarray(wq_p[j * QF:(j + 1) * QF].T.astype(bf16))
        wkTc = np.ascontiguousarray(wk_p[j * KF:(j + 1) * KF].T.astype(bf16))
        wvTc = np.ascontiguousarray(wv_n[j * KF:(j + 1) * KF].T.astype(bf16))
        wo_own = wo_n[j * COLS:(j + 1) * COLS].T.astype(bf16)
        if ag_mode in ('full8', 'full8s', 'single8'):
            woTc = np.zeros((2 * HIDDEN, COLS), dtype=bf16)
            woTc[b * HIDDEN:(b + 1) * HIDDEN] = wo_own  # own-batch rows only
        else:
            woTc = np.ascontiguousarray(wo_own)
        in_maps.append({
            "xT": xTc, "wqT": wqTc, "wkT": wkTc, "wvT": wvTc, "woT": woTc,
            "cosT": cosT.astype(bf16), "sinTs": sinTs.astype(bf16),
            "swp": swp.astype(bf16), "msk": msk.astype(bf16),
        })
    return in_maps


def kernel(x, freqs_cis, wq, wk, wv, wo):
    global LAST_EXEC_NS, LAST_RESULTS
    T = x.shape[1]
    nc = _get_nc(T)
    in_maps = make_inputs(x, freqs_cis, wq, wk, wv, wo, T)
    trace = bool(int(os.environ.get("KERNEL_TRACE", "0")))
    res = run_bass_kernel_spmd(nc, in_maps, core_ids=list(range(NCORES)),
                               trace=trace)
    LAST_EXEC_NS = res.exec_time_ns
    LAST_RESULTS = res
    out = np.empty((B_FULL, T, HIDDEN), dtype=np.float32)
    for core in range(NCORES):
        b, j = divmod(core, NTP)
        out[b, :, j * COLS:(j + 1) * COLS] = res.results[core]["out"].T
    return out
